# revision 1
# baseline (speedup 1.0000x reference)
"""Trainium2 Bass kernel for nn_CRNLayer (CRN-weighted NetVLAD pooling).

Contract: kernel(**inputs) takes the FULL unsharded fp32 inputs and returns the
FULL (64, 32768) fp32 output. Internally: data-parallel over batch N=64 across
8 NeuronCores (8 samples/core), params replicated.

Per-core algorithm (all matmuls bf16 with fp32 PSUM accumulation), one fused
per-sample pipeline so the PE stream stays dense end-to-end:
  - x uploaded twice in bf16: channel-major (C,P) and pixel-major (P,C); the
    host transpose replaces an on-chip transpose at equal HBM cost.
  - 2x2 avg-pool on VectorE; the 0.25 is folded into the conv weights.
  - the 3 convs (3x3/5x5/7x7 on the pooled map) run as tap-accumulated
    matmuls over a zero-padded (22x22) pooled map, grouped into rings
    (union channel layout [conv7|pad|conv5|conv3] = 96 cols) accumulating
    into one merged PSUM tile; bias+ReLU on ScalarE.
  - 1x1 'wa' conv as an M=1 matmul; bilinear 2x upsample as 16 small matmuls
    against a host-built (256->1024) interpolation matrix, per sample.
  - per-pixel L2 norms via ScalarE Square+accum on the pixel-major copy; the
    1/||x|| scale is folded into the softmax logits (scale arg of Exp) and
    into the assignment map 'a'; the VLAD "sum_p a" term is recovered with an
    extra ||x|| column appended to the pixel-major x.
  - VLAD aggregation as (P,K)^T @ (P,C+1) matmuls per sample, then a batched
    intra + global L2 normalization tail.
"""
import sys
from contextlib import ExitStack

import numpy as np
import ml_dtypes

try:
    import concourse.bass as bass  # noqa: F401
except ImportError:
    sys.path.insert(0, "/opt/trn_rl_repo")

import concourse.bass as bass
import concourse.mybir as mybir
import concourse.tile as tile
from concourse import bacc
from concourse.bass_utils import run_bass_kernel_spmd

BF16 = ml_dtypes.bfloat16
F32 = np.float32

N, C, W, H, K = 64, 512, 32, 32, 64
P = W * H            # 1024 pixels
Q = 256              # pooled pixels (16x16)
NCORES = 8
SPC = N // NCORES    # samples per core
EPS = 1e-12
OC = 96              # padded out-ch layout [conv7(0:20) | pad(20:32) | conv5(32:64) | conv3(64:96)]

_TAPS = [(ty, tx) for ty in range(-3, 4) for tx in range(-3, 4)]
_R1 = [(ty, tx) for (ty, tx) in _TAPS if max(abs(ty), abs(tx)) <= 1]
_R2 = [(ty, tx) for (ty, tx) in _TAPS if max(abs(ty), abs(tx)) == 2]
_R3 = [(ty, tx) for (ty, tx) in _TAPS if max(abs(ty), abs(tx)) == 3]


def _tap_index(ty, tx):
    return (ty + 3) * 7 + (tx + 3)


def upsample_matrix_16_to_32():
    """1D bilinear (align_corners=False) 16->32 resize matrix, jax semantics."""
    R = np.zeros((32, 16), np.float64)
    for i in range(32):
        pos = (i + 0.5) / 2.0 - 0.5
        lo = int(np.floor(pos))
        f = pos - lo
        tot = (1.0 - f) * (0 <= lo < 16) + f * (0 <= lo + 1 < 16)
        if 0 <= lo < 16:
            R[i, lo] = (1.0 - f) / tot
        if 0 <= lo + 1 < 16:
            R[i, lo + 1] = f / tot
    return R


# --------------------------------------------------------------------------
# device program
# --------------------------------------------------------------------------

def build_nc():
    dt = mybir.dt
    nc = bacc.Bacc("TRN2", target_bir_lowering=False, debug=False)

    d = {}
    d["xcm"] = nc.dram_tensor("xcm", [SPC, C, P], dt.bfloat16, kind="ExternalInput")
    d["xpm"] = nc.dram_tensor("xpm", [SPC, P, C], dt.bfloat16, kind="ExternalInput")
    d["wtaps"] = nc.dram_tensor("wtaps", [128, 4, 49, OC], dt.bfloat16, kind="ExternalInput")
    d["wvt"] = nc.dram_tensor("wvt", [128, 4, K], dt.bfloat16, kind="ExternalInput")
    d["bias84"] = nc.dram_tensor("bias84", [OC, 1], dt.float32, kind="ExternalInput")
    d["war"] = nc.dram_tensor("war", [OC, 1], dt.bfloat16, kind="ExternalInput")
    d["bup"] = nc.dram_tensor("bup", [128, 2, P], dt.bfloat16, kind="ExternalInput")
    d["cent"] = nc.dram_tensor("cent", [K, C], dt.float32, kind="ExternalInput")
    d["ba"] = nc.dram_tensor("ba", [1, 1], dt.float32, kind="ExternalInput")
    d["out"] = nc.dram_tensor("out", [SPC, K, C], dt.float32, kind="ExternalOutput")
    import os
    if os.environ.get("KDBG"):
        d["dbg_relu"] = nc.dram_tensor("dbg_relu", [128, Q], dt.float32, kind="ExternalOutput")
        d["dbg_mmup"] = nc.dram_tensor("dbg_mmup", [128, 8], dt.float32, kind="ExternalOutput")
        d["dbg_mmq"] = nc.dram_tensor("dbg_mmq", [1, Q], dt.float32, kind="ExternalOutput")

    with tile.TileContext(nc) as tc:
        _emit(nc, tc, d)
    nc.compile()
    return nc


def _emit(nc, tc, d):
    dt = mybir.dt
    fp = dt.float32
    bf = dt.bfloat16
    AF = mybir.ActivationFunctionType
    OP = mybir.AluOpType
    PSUM = bass.MemorySpace.PSUM

    ctx = ExitStack()
    consts = ctx.enter_context(tc.tile_pool(name="consts", bufs=1))
    persist = ctx.enter_context(tc.tile_pool(name="persist", bufs=1))
    work = ctx.enter_context(tc.tile_pool(name="work", bufs=2))
    small = ctx.enter_context(tc.tile_pool(name="small", bufs=4))
    ps = ctx.enter_context(tc.tile_pool(name="ps", bufs=2, space=PSUM))

    # ---- early load: sample 0's channel-major x, then conv weights per-chunk
    # so the first convs can start ~7us in (right as the PE warm-up ends) ----
    xcm0 = work.tile([128, 4, P], bf, tag="xcm")
    nc.sync.dma_start(
        out=xcm0, in_=d["xcm"][0].rearrange("(cc cp) p -> cp cc p", cp=128))

    # ---- constants ----
    wtaps = consts.tile([128, 4, 49, OC], bf)
    for cc in range(4):
        nc.sync.dma_start(out=wtaps[:, cc], in_=d["wtaps"][:, cc])
    wvt = consts.tile([128, 4, K], bf)
    nc.sync.dma_start(out=wvt, in_=d["wvt"][:])
    bias84 = consts.tile([OC, 1], fp)
    nc.sync.dma_start(out=bias84, in_=d["bias84"][:])
    war = consts.tile([OC, 1], bf)
    nc.sync.dma_start(out=war, in_=d["war"][:])
    bup = consts.tile([128, 2, P], bf)
    nc.sync.dma_start(out=bup, in_=d["bup"][:])
    cent = consts.tile([K, C], fp)
    nc.sync.dma_start(out=cent, in_=d["cent"][:])
    ba_bc = consts.tile([128, 1], fp)
    ba_ap = d["ba"][:]
    nc.sync.dma_start(
        out=ba_bc,
        in_=bass.AP(tensor=ba_ap.tensor, offset=ba_ap.offset,
                    ap=[[0, 128], ba_ap.ap[1]]))
    zeros = consts.tile([128, 1], fp)
    nc.vector.memset(zeros, 0.0)
    onesk = consts.tile([K, 1], fp)
    nc.vector.memset(onesk, 1.0)
    onesrow = consts.tile([1, K], fp)
    nc.vector.memset(onesrow, 1.0)

    # ---- PE warm-up: ~7us of junk matmuls so HAM unthrottles before the
    # first real convs (they otherwise run at 1.2 GHz for ~3.4us+) ----
    wj = consts.tile([128, 128], bf)
    nc.vector.memset(wj, 0.0)
    xj = consts.tile([128, 512], bf)
    nc.vector.memset(xj, 0.0)
    for i in range(16):
        wps = ps.tile([128, 512], fp, tag="rg0", bufs=1)
        nc.tensor.matmul(wps, wj, xj, start=True, stop=True)

    # ---- persistent state (consumed by the batched tail) ----
    xpm = persist.tile([128, SPC, 8, C + 1], bf)      # pixel-major x + ||x|| col
    vlad_all = persist.tile([K, SPC, C], fp)
    q2_all = persist.tile([K, SPC], fp)

    # =====================  per-sample fused pipeline  =====================
    for s in range(SPC):
        # loads
        if s == 0:
            xcm = xcm0
        else:
            xcm = work.tile([128, 4, P], bf, tag="xcm")
            nc.sync.dma_start(
                out=xcm, in_=d["xcm"][s].rearrange("(cc cp) p -> cp cc p", cp=128))
        nc.sync.dma_start(
            out=xpm[:, s, :, 0:C],
            in_=d["xpm"][s].rearrange("(pc pp) c -> pp pc c", pp=128))

        # 2x2 sum-pool into padded map (0.25 folded into weights)
        mpad = work.tile([128, 4, 22, 22], bf, tag="mpad")
        nc.gpsimd.memset(mpad, 0.0)
        for cc in range(4):
            xv = xcm[:, cc, :].rearrange("p (a b t) -> p a b t", b=16, t=2)
            ta = work.tile([128, 32, 16], bf, tag="ta")
            nc.vector.tensor_tensor(ta, xv[:, :, :, 0], xv[:, :, :, 1], op=OP.add)
            tv = ta.rearrange("p (y u) b -> p y u b", u=2)
            nc.vector.tensor_tensor(
                mpad[:, cc, 3:19, 3:19], tv[:, :, 0, :], tv[:, :, 1, :], op=OP.add)

        # convs: 32x32 PE-subtile packing. Each (row-group rg = 32-channel
        # sub-chunk of C, col-slot cs = 32 output channels) is an independent
        # 32x32 tile; up to 16 run concurrently in the PE array. Row groups
        # accumulate into separate PSUM banks (concurrent same-element
        # accumulation across sub-arrays is not safe), combined on DVE after.
        rgs = [ps.tile([128, Q], fp, tag=f"rg{g}", bufs=1, name=f"rg{g}") for g in range(4)]

        def win32(rg, cc, ty, tx):
            return mpad[32 * rg:32 * rg + 32, cc, 3 + ty:19 + ty, 3 + tx:19 + tx]

        order = [(_R1[0], OC)]
        order += [(t, 20) for t in _R3] + [(t, 64) for t in _R2]
        order += [(t, OC) for t in _R1[1:]]
        ntap = len(order)
        for cc in range(4):
            for ti, ((ty, tx), ncol) in enumerate(order):
                for rg in range(4):
                    for cs in range((ncol + 31) // 32):
                        w = min(32, ncol - 32 * cs)
                        nc.tensor.matmul(
                            rgs[rg][32 * cs:32 * cs + w, :],
                            wtaps[32 * rg:32 * rg + 32, cc,
                                  _tap_index(ty, tx), 32 * cs:32 * cs + w],
                            win32(rg, cc, ty, tx),
                            start=(cc == 0 and ti == 0),
                            stop=(cc == 3 and ti == ntap - 1),
                            skip_group_check=True,
                            tile_position=(32 * rg, 32 * cs))
        # combine the 4 row-group banks (<=1 PSUM operand per DVE op)
        scr2 = work.tile([128, Q], fp, tag="scr2")
        nc.scalar.copy(scr2[0:OC, :], rgs[0][0:OC, :])
        nc.vector.tensor_tensor(scr2[0:OC, :], scr2[0:OC, :], rgs[1][0:OC, :], op=OP.add)
        nc.vector.tensor_tensor(scr2[0:OC, :], scr2[0:OC, :], rgs[2][0:OC, :], op=OP.add)
        nc.vector.tensor_tensor(scr2[0:OC, :], scr2[0:OC, :], rgs[3][0:OC, :], op=OP.add)

        # bias+relu
        relu84 = work.tile([128, Q], bf, tag="relu84")
        nc.scalar.activation(relu84[0:OC, :], scr2[0:OC, :], AF.Relu,
                             bias=bias84[0:OC, :])

        # 1x1 'wa' conv: mm_q = wa . relu84   (M=1 matmul)
        mmps = ps.tile([1, Q], fp, tag="smallps", bufs=2)
        nc.tensor.matmul(mmps, war[0:OC, :], relu84[0:OC, :], start=True, stop=True)
        mmq_s = work.tile([1, Q], bf, tag="mmq_s")
        nc.scalar.copy(mmq_s, mmps)

        # scatter mm_q to (q-partitions, qc) for the upsample matmuls
        mmqt_s = work.tile([128, 2], bf, tag="mmqt_s")
        for qc in range(2):
            nc.sync.dma_start(out=mmqt_s[:, qc:qc + 1],
                              in_=mmq_s[:, 128 * qc:128 * (qc + 1)])

        # bilinear upsample: mmup_s[p] = sum_q B[p,q] mm_q[q]  (+ba)
        upB = ps.tile([128, 8], fp, tag="smallps", bufs=2)
        for pc in range(8):
            nc.tensor.matmul(upB[:, pc:pc + 1], bup[:, 0, 128 * pc:128 * (pc + 1)],
                             mmqt_s[:, 0:1], start=True, stop=False,
                             skip_group_check=True)
            nc.tensor.matmul(upB[:, pc:pc + 1], bup[:, 1, 128 * pc:128 * (pc + 1)],
                             mmqt_s[:, 1:2], start=False, stop=True,
                             skip_group_check=True)
        mmup_s = work.tile([128, 8], fp, tag="mmup_s")
        nc.vector.tensor_scalar(mmup_s, upB, ba_bc, None, op0=OP.add)

        # assignment logits: (x_cm chunk)^T @ wvT -> (128p, 64)
        logits_s = work.tile([128, 8, K], bf, tag="logits_s")
        for pc in range(8):
            saps = ps.tile([128, K], fp, tag="smallps", bufs=2)
            for cc in range(4):
                nc.tensor.matmul(saps, xcm[:, cc, 128 * pc:128 * (pc + 1)],
                                 wvt[:, cc, :], start=(cc == 0), stop=(cc == 3))
            nc.scalar.copy(logits_s[:, pc, :], saps)

        # per-pixel norms from pixel-major x
        nrm2 = small.tile([128, 8], fp, tag="nrm2")
        for pc in range(8):
            sqscr = work.tile([128, C], bf, tag="sqscr")
            nc.scalar.activation(sqscr, xpm[:, s, pc, 0:C], AF.Square,
                                 bias=zeros, accum_out=nrm2[:, pc:pc + 1])
        nrm = small.tile([128, 8], fp, tag="nrm")
        nc.scalar.activation(nrm, nrm2, AF.Sqrt, bias=zeros)
        nc.vector.tensor_copy(xpm[:, s, :, C:C + 1], nrm)
        s_s = small.tile([128, 8], fp, tag="s_s")
        nc.vector.tensor_scalar(nrm, nrm, EPS, None, op0=OP.max)
        nc.vector.reciprocal(s_s, nrm)

        # softmax * CRN weighting -> a
        a_s = work.tile([128, 8, K], bf, tag="a_s")
        for pc in range(8):
            e_sb = work.tile([128, K], bf, tag="e_sb")
            se = small.tile([128, 1], fp, tag="se")
            nc.scalar.activation(e_sb, logits_s[:, pc, :], AF.Exp,
                                 bias=zeros, scale=s_s[:, pc:pc + 1],
                                 accum_out=se)
            rse = small.tile([128, 1], fp, tag="rse")
            nc.vector.reciprocal(rse, se)
            gcol = small.tile([128, 1], fp, tag="gcol")
            nc.vector.tensor_tensor(gcol, mmup_s[:, pc:pc + 1],
                                    s_s[:, pc:pc + 1], op=OP.mult)
            nc.vector.tensor_scalar(a_s[:, pc, :], e_sb, rse, gcol,
                                    op0=OP.mult, op1=OP.mult)

        if s == 0 and "dbg_relu" in d:
            dbr = work.tile([128, Q], fp, tag="dbr")
            nc.vector.memset(dbr, 0.0)
            nc.vector.tensor_copy(dbr[0:OC, :], relu84[0:OC, :])
            nc.sync.dma_start(out=d["dbg_relu"][:], in_=dbr)
            nc.sync.dma_start(out=d["dbg_mmup"][:], in_=mmup_s)
            dbq = work.tile([1, Q], fp, tag="dbq")
            nc.vector.tensor_copy(dbq, mmq_s)
            nc.sync.dma_start(out=d["dbg_mmq"][:], in_=dbq)

        # VLAD GEMMs + centroid subtraction + Square+accum
        vlps = ps.tile([K, C], fp, tag="vlps")
        asps = ps.tile([K, 1], fp, tag="smallps", bufs=2)
        for pc in range(8):
            nc.tensor.matmul(vlps, a_s[:, pc, :], xpm[:, s, pc, 0:C],
                             start=(pc == 0), stop=(pc == 7))
            nc.tensor.matmul(asps, a_s[:, pc, :], xpm[:, s, pc, C:C + 1],
                             start=(pc == 0), stop=(pc == 7))
        asum = small.tile([K, 1], fp, tag="asum")
        nc.vector.tensor_copy(asum, asps)
        scr64 = work.tile([K, C], fp, tag="scr64")
        nc.vector.tensor_scalar(scr64, cent, asum, None, op0=OP.mult)
        nc.vector.tensor_tensor(vlad_all[:, s, :], vlps, scr64, op=OP.subtract)
        sqs2 = work.tile([K, C], bf, tag="sqs2")
        nc.scalar.activation(sqs2, vlad_all[:, s, :], AF.Square,
                             bias=zeros[0:K, :], accum_out=q2_all[:, s:s + 1])

    # =====================  batched normalization tail  =====================
    nkk = small.tile([K, SPC], fp, tag="nkk")
    nc.scalar.activation(nkk, q2_all, AF.Sqrt, bias=zeros[0:K, :])
    nc.vector.tensor_scalar(nkk, nkk, EPS, None, op0=OP.max)
    rrk = small.tile([K, SPC], fp, tag="rrk")
    nc.vector.reciprocal(rrk, nkk)
    ttk = small.tile([K, SPC], fp, tag="ttk")
    nc.vector.tensor_tensor(ttk, q2_all, rrk, op=OP.mult)
    nc.vector.tensor_tensor(ttk, ttk, rrk, op=OP.mult)
    gnps = ps.tile([1, SPC], fp, tag="smallps", bufs=2)
    nc.tensor.matmul(gnps, onesk, ttk, start=True, stop=True)
    gs = small.tile([1, SPC], fp, tag="gs")
    nc.scalar.activation(gs, gnps, AF.Sqrt, bias=zeros[0:1, :])
    nc.vector.tensor_scalar(gs, gs, EPS, None, op0=OP.max)
    gr = small.tile([1, SPC], fp, tag="gr")
    nc.vector.reciprocal(gr, gs)
    gbps = ps.tile([K, SPC], fp, tag="smallps", bufs=2)
    nc.tensor.matmul(gbps, onesrow, gr, start=True, stop=True)
    rfin = small.tile([K, SPC], fp, tag="rfin")
    nc.vector.tensor_tensor(rfin, rrk, gbps, op=OP.mult)
    for s in range(SPC):
        outf = work.tile([K, C], fp, tag="outf")
        nc.vector.tensor_scalar(outf, vlad_all[:, s, :], rfin[:, s:s + 1], None,
                                op0=OP.mult)
        nc.sync.dma_start(out=d["out"][s], in_=outf)

    ctx.close()


# --------------------------------------------------------------------------
# host side
# --------------------------------------------------------------------------

def prep_params(w1, b1, w2, b2, w3, b3, wa, ba, wv, centroids):
    """Build the replicated device parameter tensors (numpy, host-side)."""
    wtaps = np.zeros((128, 4, 49, OC), BF16)
    w1q = (w1 * 0.25).astype(F32)
    w2q = (w2 * 0.25).astype(F32)
    w3q = (w3 * 0.25).astype(F32)
    for ty in range(-3, 4):
        for tx in range(-3, 4):
            t = _tap_index(ty, tx)
            m = np.zeros((512, OC), F32)
            m[:, 0:20] = w3q[:, :, ty + 3, tx + 3].T
            if max(abs(ty), abs(tx)) <= 2:
                m[:, 32:64] = w2q[:, :, ty + 2, tx + 2].T
            if max(abs(ty), abs(tx)) <= 1:
                m[:, 64:96] = w1q[:, :, ty + 1, tx + 1].T
            wtaps[:, :, t, :] = (
                m.reshape(4, 128, OC).transpose(1, 0, 2).astype(BF16))
    wvt = wv.T.reshape(4, 128, K).transpose(1, 0, 2).astype(BF16)
    z12 = np.zeros(12, F32)
    bias84 = np.concatenate([b3, z12, b2, b1]).astype(F32)[:, None]
    war = np.concatenate([wa[0, 64:84, 0, 0], z12, wa[0, 32:64, 0, 0],
                          wa[0, 0:32, 0, 0]]).astype(BF16)[:, None]
    R = upsample_matrix_16_to_32()
    B = np.kron(R, R)                                       # (1024, 256)
    bup = B.T.reshape(2, 128, P).transpose(1, 0, 2).astype(BF16)
    return {
        "wtaps": wtaps,
        "wvt": np.ascontiguousarray(wvt),
        "bias84": bias84,
        "war": war,
        "bup": np.ascontiguousarray(bup),
        "cent": centroids.astype(F32),
        "ba": ba.astype(F32).reshape(1, 1),
    }


_NC_CACHE = None


def _get_nc():
    global _NC_CACHE
    if _NC_CACHE is None:
        _NC_CACHE = build_nc()
    return _NC_CACHE


def make_in_maps(x, params):
    x_bf = x.reshape(N, C, P).astype(BF16)
    in_maps = []
    for core in range(NCORES):
        xs = x_bf[core * SPC:(core + 1) * SPC]
        in_maps.append({
            "xcm": np.ascontiguousarray(xs),
            "xpm": np.ascontiguousarray(xs.transpose(0, 2, 1)),
            **params,
        })
    return in_maps


def kernel(x, w1, b1, w2, b2, w3, b3, wa, ba, wv, centroids, **_ignored):
    x = np.asarray(x, F32)
    params = prep_params(
        np.asarray(w1, F32), np.asarray(b1, F32), np.asarray(w2, F32),
        np.asarray(b2, F32), np.asarray(w3, F32), np.asarray(b3, F32),
        np.asarray(wa, F32), np.asarray(ba, F32), np.asarray(wv, F32),
        np.asarray(centroids, F32))
    nc = _get_nc()
    res = run_bass_kernel_spmd(nc, make_in_maps(x, params), list(range(NCORES)))
    out = np.concatenate([r["out"].reshape(SPC, K * C) for r in res.results], axis=0)
    return out.astype(F32)



# revision 2
# speedup vs baseline: 2.0332x; 2.0332x over previous
"""Trainium2 Bass kernel for nn_CRNLayer (CRN-weighted NetVLAD pooling).

Contract: kernel(**inputs) takes the FULL unsharded fp32 inputs and returns the
FULL (64, 32768) fp32 output. Internally: data-parallel over batch N=64 across
8 NeuronCores (8 samples/core), params replicated.

v2 pipeline (vs the 32x32-packed baseline):
  - convs as dense 128-contraction matmuls: per (cc, tap) one matmul
    lhsT=W[128c, ncol<=96] @ window[128c, 256px], all 196 accumulating into a
    single PSUM bank per sample. No DVE row-group combine; bias+ReLU reads
    PSUM directly. The 32x32-packed variant was NX-issue-bound at ~45ns/MM;
    dense matmuls stream at ~107ns for 12x more work each.
  - per-pixel 1/||x|| via Exp(-0.5*Ln(nrm2)) so the whole kernel uses ONE
    activation table set (natural_log_exp_and_others: exp/ln/relu/square) --
    the baseline's per-sample Sqrt<->Exp alternation forced 2 table reloads
    (~5.3us) per sample.
  - the 'wa' 1x1 conv is computed pre-transposed on the PE
    (mmqt[q,1] = relu84[:,q128]^T @ war) instead of an M=1 matmul followed by
    an SBUF->SBUF DMA scatter on the critical path.
  - bilinear upsample matmuls batched over sample pairs (N=2).
  - software-pipelined emission: PE stream per pair is
    [convs a][vlad prev][logits a][convs b][logits b][mmqt][upB] so ScalarE/
    DVE softmax work always overlaps matmuls and the PE (and HAM) stays warm.
"""
import sys
from contextlib import ExitStack

import numpy as np
import ml_dtypes

try:
    import concourse.bass as bass  # noqa: F401
except ImportError:
    sys.path.insert(0, "/opt/trn_rl_repo")

import concourse.bass as bass
import concourse.mybir as mybir
import concourse.tile as tile
from concourse import bacc
from concourse.bass_utils import run_bass_kernel_spmd

BF16 = ml_dtypes.bfloat16
F32 = np.float32

N, C, W, H, K = 64, 512, 32, 32, 64
P = W * H            # 1024 pixels
Q = 256              # pooled pixels (16x16)
NCORES = 8
SPC = N // NCORES    # samples per core
OC = 96              # padded out-ch layout [conv7(0:20) | pad(20:32) | conv5(32:64) | conv3(64:96)]

_TAPS = [(ty, tx) for ty in range(-3, 4) for tx in range(-3, 4)]
_R1 = [(ty, tx) for (ty, tx) in _TAPS if max(abs(ty), abs(tx)) <= 1]
_R2 = [(ty, tx) for (ty, tx) in _TAPS if max(abs(ty), abs(tx)) == 2]
_R3 = [(ty, tx) for (ty, tx) in _TAPS if max(abs(ty), abs(tx)) == 3]

# conv tap order: an R1 tap first so the start=True matmul covers all 96 rows
CONV_ORDER = [(_R1[0], OC)]
CONV_ORDER += [(t, 20) for t in _R3] + [(t, 64) for t in _R2]
CONV_ORDER += [(t, OC) for t in _R1[1:]]


def _tap_index(ty, tx):
    return (ty + 3) * 7 + (tx + 3)


def upsample_matrix_16_to_32():
    """1D bilinear (align_corners=False) 16->32 resize matrix, jax semantics."""
    R = np.zeros((32, 16), np.float64)
    for i in range(32):
        pos = (i + 0.5) / 2.0 - 0.5
        lo = int(np.floor(pos))
        f = pos - lo
        tot = (1.0 - f) * (0 <= lo < 16) + f * (0 <= lo + 1 < 16)
        if 0 <= lo < 16:
            R[i, lo] = (1.0 - f) / tot
        if 0 <= lo + 1 < 16:
            R[i, lo + 1] = f / tot
    return R


# --------------------------------------------------------------------------
# device program
# --------------------------------------------------------------------------

def build_nc():
    dt = mybir.dt
    nc = bacc.Bacc("TRN2", target_bir_lowering=False, debug=False)

    d = {}
    d["xcm"] = nc.dram_tensor("xcm", [SPC, C, P], dt.bfloat16, kind="ExternalInput")
    d["xpm"] = nc.dram_tensor("xpm", [SPC, P, C], dt.bfloat16, kind="ExternalInput")
    d["wtaps"] = nc.dram_tensor("wtaps", [128, 4, 49, OC], dt.bfloat16, kind="ExternalInput")
    d["wvt"] = nc.dram_tensor("wvt", [128, 4, K], dt.bfloat16, kind="ExternalInput")
    d["bias84"] = nc.dram_tensor("bias84", [OC, 1], dt.float32, kind="ExternalInput")
    d["war"] = nc.dram_tensor("war", [OC, 1], dt.bfloat16, kind="ExternalInput")
    d["bup"] = nc.dram_tensor("bup", [128, 2, P], dt.bfloat16, kind="ExternalInput")
    d["cent"] = nc.dram_tensor("cent", [K, C], dt.float32, kind="ExternalInput")
    d["ba"] = nc.dram_tensor("ba", [1, 1], dt.float32, kind="ExternalInput")
    d["out"] = nc.dram_tensor("out", [SPC, K, C], dt.float32, kind="ExternalOutput")

    with tile.TileContext(nc) as tc:
        _emit(nc, tc, d)
    nc.compile()
    return nc


def _emit(nc, tc, d):
    dt = mybir.dt
    fp = dt.float32
    bf = dt.bfloat16
    AF = mybir.ActivationFunctionType
    OP = mybir.AluOpType
    PSUM = bass.MemorySpace.PSUM

    ctx = ExitStack()
    consts = ctx.enter_context(tc.tile_pool(name="consts", bufs=1))
    persist = ctx.enter_context(tc.tile_pool(name="persist", bufs=1))
    xin = ctx.enter_context(tc.tile_pool(name="xin", bufs=4))
    work = ctx.enter_context(tc.tile_pool(name="work", bufs=2))
    small = ctx.enter_context(tc.tile_pool(name="small", bufs=4))
    ps = ctx.enter_context(tc.tile_pool(name="ps", bufs=2, space=PSUM))

    # ---- early load: sample 0/1 inputs first so compute can start ASAP ----
    xcm_t = {}
    xpm_t = {}

    def load_xcm(s):
        xcm_t[s] = xin.tile([128, 4, P], bf, tag="xcm", name=f"xcm{s}")
        nc.sync.dma_start(
            out=xcm_t[s], in_=d["xcm"][s].rearrange("(cc cp) p -> cp cc p", cp=128))

    def load_xpm(s):
        xpm_t[s] = xin.tile([128, 8, C + 1], bf, tag="xpm", name=f"xpm{s}")
        nc.sync.dma_start(
            out=xpm_t[s][:, :, 0:C],
            in_=d["xpm"][s].rearrange("(pc pp) c -> pp pc c", pp=128))

    load_xcm(0)

    # ---- constants ----
    wtaps = consts.tile([128, 4, 49, OC], bf)
    for cc in range(4):
        nc.sync.dma_start(out=wtaps[:, cc], in_=d["wtaps"][:, cc])
    load_xpm(0)
    load_xcm(1)
    load_xpm(1)
    wvt = consts.tile([128, 4, K], bf)
    nc.sync.dma_start(out=wvt, in_=d["wvt"][:])
    bias84 = consts.tile([OC, 1], fp)
    nc.sync.dma_start(out=bias84, in_=d["bias84"][:])
    war = consts.tile([OC, 1], bf)
    nc.sync.dma_start(out=war, in_=d["war"][:])
    bup = consts.tile([128, 2, P], bf)
    nc.sync.dma_start(out=bup, in_=d["bup"][:])
    cent = consts.tile([K, C], fp)
    nc.sync.dma_start(out=cent, in_=d["cent"][:])
    ba_bc = consts.tile([128, 1], fp)
    ba_ap = d["ba"][:]
    nc.sync.dma_start(
        out=ba_bc,
        in_=bass.AP(tensor=ba_ap.tensor, offset=ba_ap.offset,
                    ap=[[0, 128], ba_ap.ap[1]]))
    zeros = consts.tile([128, 1], fp)
    nc.vector.memset(zeros, 0.0)
    onesk = consts.tile([K, 1], fp)
    nc.vector.memset(onesk, 1.0)
    onesrow = consts.tile([1, K], fp)
    nc.vector.memset(onesrow, 1.0)

    # ---- PE warm-up: ~7us of junk matmuls so HAM unthrottles before the
    # first real convs ----
    wj = consts.tile([128, 128], bf)
    nc.vector.memset(wj, 0.0)
    xj = consts.tile([128, 512], bf)
    nc.vector.memset(xj, 0.0)
    for i in range(16):
        wps = ps.tile([128, 512], fp, tag="convps", bufs=2, name=f"warm{i}")
        nc.tensor.matmul(wps, wj, xj, start=True, stop=True)

    # ---- persistent state (consumed by the batched tail) ----
    vlad_all = persist.tile([K, SPC, C], fp)
    q2_all = persist.tile([K, SPC], fp)

    # ---------------- per-sample emission helpers ----------------
    mpad_t = {}

    def pool(s):
        """2x2 sum-pool xcm(s) into zero-padded 22x22 map (0.25 in weights)."""
        mpad_t[s] = xin.tile([128, 4, 22, 22], bf, tag="mpad", name=f"mpad{s}")
        mpad = mpad_t[s]
        nc.gpsimd.memset(mpad, 0.0)
        for cc in range(4):
            xv = xcm_t[s][:, cc, :].rearrange("p (a b t) -> p a b t", b=16, t=2)
            ta = work.tile([128, 32, 16], bf, tag="ta", name=f"ta{s}{cc}")
            nc.vector.tensor_tensor(ta, xv[:, :, :, 0], xv[:, :, :, 1], op=OP.add)
            tv = ta.rearrange("p (y u) b -> p y u b", u=2)
            nc.vector.tensor_tensor(
                mpad[:, cc, 3:19, 3:19], tv[:, :, 0, :], tv[:, :, 1, :], op=OP.add)

    nrm2_t = {}
    ss_t = {}

    def norms(s):
        """nrm2 = sum_c x^2 per pixel; s_s = 1/||x||; ||x|| -> xpm col C.

        Split squares: ScalarE does pc 0..3 (Square+accum), DVE does pc 4..7.
        1/sqrt via Exp(-0.5*Ln()) to stay inside one activation table set.
        """
        xpm = xpm_t[s]
        nrm2_t[s] = small.tile([128, 8], fp, tag="nrm2", name=f"nrm2{s}")
        nrm2 = nrm2_t[s]
        for pc in range(4):
            sqscr = work.tile([128, C], bf, tag="sqscr", name=f"sqs{s}{pc}")
            nc.scalar.activation(sqscr, xpm[:, pc, 0:C], AF.Square,
                                 bias=zeros, accum_out=nrm2[:, pc:pc + 1])
        sq2 = work.tile([128, 4, C], bf, tag="sq2", name=f"sq2{s}")
        nc.vector.tensor_tensor(sq2, xpm[:, 4:8, 0:C], xpm[:, 4:8, 0:C],
                                op=OP.mult)
        nc.vector.tensor_reduce(nrm2[:, 4:8], sq2, axis=mybir.AxisListType.X,
                                op=OP.add)
        lgn = small.tile([128, 8], fp, tag="lgn", name=f"lgn{s}")
        nc.scalar.activation(lgn, nrm2, AF.Ln, bias=zeros)
        ss_t[s] = small.tile([128, 8], fp, tag="ss", name=f"ss{s}")
        nc.scalar.activation(ss_t[s], lgn, AF.Exp, bias=zeros, scale=-0.5)
        # ||x|| = nrm2 * (1/||x||)  -> extra VLAD column
        nc.vector.tensor_tensor(xpm[:, :, C:C + 1], nrm2, ss_t[s], op=OP.mult)

    def convs(s):
        """196 dense matmuls accumulating into one PSUM bank [96, 256]."""
        convps = ps.tile([OC, Q], fp, tag="convps", bufs=2,
                         padded_shape=[128, 512], name=f"convps{s}")
        mpad = mpad_t[s]
        nlast = len(CONV_ORDER) - 1
        for cc in range(4):
            for ti, ((ty, tx), ncol) in enumerate(CONV_ORDER):
                nc.tensor.matmul(
                    convps[0:ncol, :],
                    wtaps[:, cc, _tap_index(ty, tx), 0:ncol],
                    mpad[:, cc, 3 + ty:19 + ty, 3 + tx:19 + tx],
                    start=(cc == 0 and ti == 0),
                    stop=(cc == 3 and ti == nlast),
                    skip_group_check=True)
        return convps

    relu_t = {}

    def relu(s, convps):
        relu_t[s] = work.tile([OC, Q], bf, tag="relu84", name=f"relu{s}")
        nc.scalar.activation(relu_t[s], convps[0:OC, :], AF.Relu,
                             bias=bias84[0:OC, :])

    lgs_t = {}

    def logits(s):
        """(x_cm chunk)^T @ wvT -> (128p, 64); 1/||x|| folded in at evac."""
        lgs_t[s] = work.tile([128, 8, K], bf, tag="lgs", name=f"lgs{s}")
        for pc in range(8):
            saps = ps.tile([128, K], fp, tag="smallps", bufs=4, name=f"sa{s}{pc}")
            for cc in range(4):
                nc.tensor.matmul(saps, xcm_t[s][:, cc, 128 * pc:128 * (pc + 1)],
                                 wvt[:, cc, :], start=(cc == 0), stop=(cc == 3))
            nc.vector.tensor_scalar(lgs_t[s][:, pc, :], saps,
                                    ss_t[s][:, pc:pc + 1], None, op0=OP.mult)

    def mmqt(s, mmqt2, j):
        """wa 1x1 conv, output already transposed: mmqt[q,1] over 2 q-chunks."""
        mqps = ps.tile([128, 2], fp, tag="smallps", bufs=4, name=f"mq{s}")
        for qc in range(2):
            nc.tensor.matmul(mqps[:, qc:qc + 1],
                             relu_t[s][0:OC, 128 * qc:128 * (qc + 1)],
                             war[0:OC, :], start=True, stop=True,
                             skip_group_check=True)
        nc.scalar.copy(mmqt2[:, :, j:j + 1], mqps)

    def upsample_pair(mmqt2, mmup2):
        """mmup[p, j] = sum_q B[p,q] mm_q[q, j] for the 2 samples at once."""
        upps = ps.tile([128, 8, 2], fp, tag="smallps", bufs=4, name="upps")
        for pc in range(8):
            for qc in range(2):
                nc.tensor.matmul(upps[:, pc, :],
                                 bup[:, qc, 128 * pc:128 * (pc + 1)],
                                 mmqt2[:, qc, :], start=(qc == 0), stop=(qc == 1),
                                 skip_group_check=True)
        nc.vector.tensor_scalar(mmup2, upps, ba_bc, None, op0=OP.add)

    a_t = {}

    def softmax(s, mmup2, j):
        """a = softmax(logits)*mm*s_s  (s_s already folded into logits)."""
        e_sb = work.tile([128, 8, K], bf, tag="esb", name=f"esb{s}")
        nc.scalar.activation(e_sb, lgs_t[s], AF.Exp, bias=zeros)
        se = small.tile([128, 8], fp, tag="se", name=f"se{s}")
        nc.vector.tensor_reduce(se, e_sb, axis=mybir.AxisListType.X, op=OP.add)
        rse = small.tile([128, 8], fp, tag="rse", name=f"rse{s}")
        nc.vector.reciprocal(rse, se)
        gcl = small.tile([128, 8], fp, tag="gcl", name=f"gcl{s}")
        nc.vector.tensor_tensor(gcl, mmup2[:, :, j], ss_t[s], op=OP.mult)
        a_t[s] = work.tile([128, 8, K], bf, tag="a_s", name=f"a{s}")
        for pc in range(8):
            nc.vector.tensor_scalar(a_t[s][:, pc, :], e_sb[:, pc, :],
                                    rse[:, pc:pc + 1], gcl[:, pc:pc + 1],
                                    op0=OP.mult, op1=OP.mult)

    def vlad(s):
        """VLAD GEMMs + centroid subtraction + Square+accum."""
        vlps = ps.tile([K, C], fp, tag="vlps", bufs=2, name=f"vl{s}")
        asps = ps.tile([K, 1], fp, tag="smallps", bufs=4, name=f"as{s}")
        xpm = xpm_t[s]
        for pc in range(8):
            nc.tensor.matmul(vlps, a_t[s][:, pc, :], xpm[:, pc, 0:C],
                             start=(pc == 0), stop=(pc == 7))
            nc.tensor.matmul(asps, a_t[s][:, pc, :], xpm[:, pc, C:C + 1],
                             start=(pc == 0), stop=(pc == 7))
        asum = small.tile([K, 1], fp, tag="asum", name=f"asum{s}")
        nc.vector.tensor_copy(asum, asps)
        scr64 = work.tile([K, C], fp, tag="scr64", name=f"scr{s}")
        nc.vector.tensor_scalar(scr64, cent, asum, None, op0=OP.mult)
        nc.vector.tensor_tensor(vlad_all[:, s, :], vlps, scr64, op=OP.subtract)
        sqs2 = work.tile([K, C], bf, tag="sqs2", name=f"sqv{s}")
        nc.scalar.activation(sqs2, vlad_all[:, s, :], AF.Square,
                             bias=zeros[0:K, :], accum_out=q2_all[:, s:s + 1])

    # =====================  pipelined pair loop  =====================
    # prologue work for pair 0
    pool(0)
    norms(0)
    pool(1)
    norms(1)

    for p in range(SPC // 2):
        a, b = 2 * p, 2 * p + 1
        # prefetch next pair's inputs
        if a + 2 < SPC:
            load_xcm(a + 2)
            load_xpm(a + 2)
            load_xcm(a + 3)
            load_xpm(a + 3)

        cps_a = convs(a)          # PE: dense conv burst for sample a
        if p > 0:
            vlad(2 * p - 2)       # PE: prev pair's VLAD (operands ready)
            vlad(2 * p - 1)
        relu(a, cps_a)            # ScalarE (runs as soon as convs(a) drain)
        logits(a)                 # PE keeps streaming; DVE evacs with 1/||x||
        # next pair's pools/norms on DVE/ScalarE while PE is busy
        if a + 2 < SPC:
            pool(a + 2)
            norms(a + 2)
        cps_b = convs(b)          # PE: conv burst for sample b
        relu(b, cps_b)
        logits(b)
        if a + 3 < SPC:
            pool(a + 3)
            norms(a + 3)

        mmqt2 = small.tile([128, 2, 2], bf, tag="mmqt2", name=f"mmqt2{p}")
        mmqt(a, mmqt2, 0)
        mmqt(b, mmqt2, 1)
        mmup2 = small.tile([128, 8, 2], fp, tag="mmup2", name=f"mmup2{p}")
        upsample_pair(mmqt2, mmup2)
        softmax(a, mmup2, 0)
        softmax(b, mmup2, 1)

    vlad(SPC - 2)
    vlad(SPC - 1)

    # =====================  batched normalization tail  =====================
    # intra-row 1/||v_k|| via Exp(-0.5 Ln(q2)); global factor via the same
    lq = small.tile([K, SPC], fp, tag="lq")
    nc.scalar.activation(lq, q2_all, AF.Ln, bias=zeros[0:K, :])
    rrk = small.tile([K, SPC], fp, tag="rrk")
    nc.scalar.activation(rrk, lq, AF.Exp, bias=zeros[0:K, :], scale=-0.5)
    ttk = small.tile([K, SPC], fp, tag="ttk")
    nc.vector.tensor_tensor(ttk, q2_all, rrk, op=OP.mult)
    nc.vector.tensor_tensor(ttk, ttk, rrk, op=OP.mult)
    gnps = ps.tile([1, SPC], fp, tag="smallps", bufs=4, name="gn")
    nc.tensor.matmul(gnps, onesk, ttk, start=True, stop=True)
    lg2 = small.tile([1, SPC], fp, tag="lg2")
    nc.scalar.activation(lg2, gnps, AF.Ln, bias=zeros[0:1, :])
    gr = small.tile([1, SPC], fp, tag="gr")
    nc.scalar.activation(gr, lg2, AF.Exp, bias=zeros[0:1, :], scale=-0.5)
    gbps = ps.tile([K, SPC], fp, tag="smallps", bufs=4, name="gb")
    nc.tensor.matmul(gbps, onesrow, gr, start=True, stop=True)
    rfin = small.tile([K, SPC], fp, tag="rfin")
    nc.vector.tensor_tensor(rfin, rrk, gbps, op=OP.mult)
    for s in range(SPC):
        outf = work.tile([K, C], fp, tag="outf", name=f"outf{s}")
        nc.vector.tensor_scalar(outf, vlad_all[:, s, :], rfin[:, s:s + 1], None,
                                op0=OP.mult)
        nc.sync.dma_start(out=d["out"][s], in_=outf)

    ctx.close()


# --------------------------------------------------------------------------
# host side
# --------------------------------------------------------------------------

def prep_params(w1, b1, w2, b2, w3, b3, wa, ba, wv, centroids):
    """Build the replicated device parameter tensors (numpy, host-side)."""
    wtaps = np.zeros((128, 4, 49, OC), BF16)
    w1q = (w1 * 0.25).astype(F32)
    w2q = (w2 * 0.25).astype(F32)
    w3q = (w3 * 0.25).astype(F32)
    for ty in range(-3, 4):
        for tx in range(-3, 4):
            t = _tap_index(ty, tx)
            m = np.zeros((512, OC), F32)
            m[:, 0:20] = w3q[:, :, ty + 3, tx + 3].T
            if max(abs(ty), abs(tx)) <= 2:
                m[:, 32:64] = w2q[:, :, ty + 2, tx + 2].T
            if max(abs(ty), abs(tx)) <= 1:
                m[:, 64:96] = w1q[:, :, ty + 1, tx + 1].T
            wtaps[:, :, t, :] = (
                m.reshape(4, 128, OC).transpose(1, 0, 2).astype(BF16))
    wvt = wv.T.reshape(4, 128, K).transpose(1, 0, 2).astype(BF16)
    z12 = np.zeros(12, F32)
    bias84 = np.concatenate([b3, z12, b2, b1]).astype(F32)[:, None]
    war = np.concatenate([wa[0, 64:84, 0, 0], z12, wa[0, 32:64, 0, 0],
                          wa[0, 0:32, 0, 0]]).astype(BF16)[:, None]
    R = upsample_matrix_16_to_32()
    B = np.kron(R, R)                                       # (1024, 256)
    bup = B.T.reshape(2, 128, P).transpose(1, 0, 2).astype(BF16)
    return {
        "wtaps": wtaps,
        "wvt": np.ascontiguousarray(wvt),
        "bias84": bias84,
        "war": war,
        "bup": np.ascontiguousarray(bup),
        "cent": centroids.astype(F32),
        "ba": ba.astype(F32).reshape(1, 1),
    }


_NC_CACHE = None


def _get_nc():
    global _NC_CACHE
    if _NC_CACHE is None:
        _NC_CACHE = build_nc()
    return _NC_CACHE


def make_in_maps(x, params):
    x_bf = x.reshape(N, C, P).astype(BF16)
    in_maps = []
    for core in range(NCORES):
        xs = x_bf[core * SPC:(core + 1) * SPC]
        in_maps.append({
            "xcm": np.ascontiguousarray(xs),
            "xpm": np.ascontiguousarray(xs.transpose(0, 2, 1)),
            **params,
        })
    return in_maps


def kernel(x, w1, b1, w2, b2, w3, b3, wa, ba, wv, centroids, **_ignored):
    x = np.asarray(x, F32)
    params = prep_params(
        np.asarray(w1, F32), np.asarray(b1, F32), np.asarray(w2, F32),
        np.asarray(b2, F32), np.asarray(w3, F32), np.asarray(b3, F32),
        np.asarray(wa, F32), np.asarray(ba, F32), np.asarray(wv, F32),
        np.asarray(centroids, F32))
    nc = _get_nc()
    res = run_bass_kernel_spmd(nc, make_in_maps(x, params), list(range(NCORES)))
    out = np.concatenate([r["out"].reshape(SPC, K * C) for r in res.results], axis=0)
    return out.astype(F32)
